# revision 60
# baseline (speedup 1.0000x reference)
"""2-layer IndRNN (diagonal recurrence) + linear head on 8 trn2 NeuronCores.

Strategy (data-parallel over batch, 32 rows/core, ONE chunk of BC=32):
  - Feature-major layout [h_inner=partition, (o, t, b)=free]; 16-t blocks.
  - GEMM-0 all-fp16 (x and W0 converted on host), one matmul per (m, block);
    PSUM->SBUF copy on Act fuses bias b0 + fp16 convert into the pre0 ring
    (2-m-tile groups when biases are zero, per-m otherwise).
  - Recurrence keeps the fp16 pre-activation state z_t in place in the pre
    ring, two DVE ops per step:
      tm  = (z_{t-1} max 0) * u   (scalar_tensor_tensor)
      z_t = tm + pre_t            (tensor_tensor add, fp16 2x mode)
  - h0 = relu(z0) -> fp8e4 ring (Act, per-4-m-tile block ops).
  - GEMM-1 in fp8e4 DoubleRow perf mode: 8 k-pair matmuls (2 k-tiles each)
    per m-tile per block; W1 pre-scaled x64 on host; the Act copy applies
    scale=1/64 + bias b1 + fp16 convert into the pre1 ring.
  - Recurrence 1 in place in the pre1 ring; head = relu(z1[T-1]) -> bf16,
    16 accumulated [128,1]x[128,BC] matmuls + lin_b bias.
Host side only reorders/shards numpy inputs; all FLOPs run on device.
"""

import numpy as np

B, T, I, H = 256, 100, 128, 2048
NCORES = 8
BL = B // NCORES            # batch rows per core
BC = BL                     # one chunk
NO = H // 128               # 16 h-tiles
NKP = NO // 2               # 8 fp8 k-pairs
TBLKS = [(0, 4), (4, 8), (12, 16), (28, 16), (44, 16), (60, 16), (76, 16),
         (92, 4), (96, 4)]
T2B = {}
for _nb, (_t0, _tb) in enumerate(TBLKS):
    for _i in range(_tb):
        T2B[_t0 + _i] = (_nb, _i)
S1 = 64.0                   # fp8 weight pre-scale for W1
FP8 = True                  # flip to False for bf16 GEMM-1 fallback

_CACHE = {}


def _build(zero_bias=False, repeat=1):
    import concourse.tile as tile
    from concourse import bacc, mybir

    f32 = mybir.dt.float32
    f16 = mybir.dt.float16
    bf16 = mybir.dt.bfloat16
    f32r = mybir.dt.float32r
    f8 = mybir.dt.float8e4
    g1dt = f8 if FP8 else bf16
    RELU = mybir.ActivationFunctionType.Relu
    IDENT = mybir.ActivationFunctionType.Identity
    DR = mybir.MatmulPerfMode.DoubleRow
    MAX = mybir.AluOpType.max
    MULT = mybir.AluOpType.mult
    # bias==0 for this problem's inputs -> wider PSUM->SBUF copies (the
    # activation bias operand is per-partition, so nonzero per-m biases
    # force per-m copies). Verified at kernel() time; nonzero falls back.
    MG = 2 if zero_bias else 1

    nc = bacc.Bacc(None, target_bir_lowering=False)

    xT_d = nc.dram_tensor("xT", [128, T, BC], f16, kind="ExternalInput")
    w0T_d = nc.dram_tensor("w0T", [128, NO, 128], f16, kind="ExternalInput")
    w1T_d = nc.dram_tensor("w1T", [128, NO, NO, 128], g1dt, kind="ExternalInput")
    u0f_d = nc.dram_tensor("u0f", [128, NO, BC], f16, kind="ExternalInput")
    u1f_d = nc.dram_tensor("u1f", [128, NO, BC], f16, kind="ExternalInput")
    b0_d = nc.dram_tensor("b0t", [128, NO], f32, kind="ExternalInput")
    b1_d = nc.dram_tensor("b1t", [128, NO], f32, kind="ExternalInput")
    lw_d = nc.dram_tensor("lwt", [128, NO], bf16, kind="ExternalInput")
    lb_d = nc.dram_tensor("lbt", [1, 1], f32, kind="ExternalInput")
    out_d = nc.dram_tensor("out", [1, BL], f32, kind="ExternalOutput")

    with tile.TileContext(nc) as tc:
        with (
            tc.tile_pool(name="const", bufs=1) as const,
            tc.tile_pool(name="p0", bufs=4) as p0p,
            tc.tile_pool(name="h0", bufs=3) as h0p,
            tc.tile_pool(name="p1", bufs=3) as p1p,
            tc.tile_pool(name="tmp", bufs=3) as tmp,
            tc.tile_pool(name="ps0", bufs=2, space="PSUM") as ps0,
            tc.tile_pool(name="ps1", bufs=2, space="PSUM") as ps1,
        ):
            xs = const.tile([128, T, BC], f16, tag="xs")
            w0T = const.tile([128, NO, 128], f16, tag="w0T")
            w1T = const.tile([128, NO, NO, 128], g1dt, tag="w1T")
            u0f = const.tile([128, NO, BC], f16, tag="u0f")
            u1f = const.tile([128, NO, BC], f16, tag="u1f")
            b0t = const.tile([128, NO], f32, tag="b0t")
            b1t = const.tile([128, NO], f32, tag="b1t")
            lwt = const.tile([128, NO], bf16, tag="lwt")
            lbt = const.tile([1, 1], f32, tag="lbt")
            outs = const.tile([1, BL], f32, tag="outs")

            # first x block + GEMM-0 weights first so the pipeline starts
            # immediately; bulk x and the large W1 stream behind them.
            t1 = TBLKS[0][1]
            nc.sync.dma_start(out=xs[:, :t1], in_=xT_d[:, :t1])
            nc.sync.dma_start(out=w0T[:], in_=w0T_d[:])
            nc.sync.dma_start(out=u0f[:], in_=u0f_d[:])
            nc.sync.dma_start(out=b0t[:], in_=b0_d[:])
            nc.sync.dma_start(out=xs[:, t1:], in_=xT_d[:, t1:])
            for kb in range(NO):
                nc.sync.dma_start(out=w1T[:, kb], in_=w1T_d[:, kb])
            nc.sync.dma_start(out=u1f[:], in_=u1f_d[:])
            nc.sync.dma_start(out=b1t[:], in_=b1_d[:])
            nc.sync.dma_start(out=lwt[:], in_=lw_d[:])
            nc.sync.dma_start(out=lbt[:], in_=lb_d[:])

            p0blk = []
            h0blk = []
            p1blk = []

            def emit_g0(nb):
                t0, TB = TBLKS[nb]
                pb = p0p.tile([128, NO, TB, BC], f16, tag="p0")
                p0blk.append(pb)
                # small ramp blocks: 4-m-tile groups fit the same 2-bank
                # PSUM footprint, halving copy count on the critical lead-in
                mg = 4 if (zero_bias and TB <= 8) else MG
                for m0 in range(0, NO, mg):
                    ps = ps0.tile([128, mg, 16 * MG // mg, BC], f32, tag="ps0")
                    for j in range(mg):
                        nc.tensor.matmul(
                            ps[:, j, :TB], w0T[:, m0 + j], xs[:, t0:t0 + TB],
                            start=True, stop=True,
                        )
                    if zero_bias:
                        nc.scalar.activation(
                            pb[:, m0:m0 + mg], ps[:, :, :TB], IDENT,
                            bias=0.0, scale=1.0,
                        )
                    else:
                        nc.scalar.activation(
                            pb[:, m0], ps[:, 0, :TB], IDENT,
                            bias=b0t[:, m0:m0 + 1], scale=1.0,
                        )

            def emit_rec(nb, blks, uf, tag):
                t0, TB = TBLKS[nb]
                for trel in range(TB):
                    t = t0 + trel
                    if t == 0:
                        continue
                    cur = blks[nb][:, :, trel]
                    pb, po = T2B[t - 1]
                    prev = blks[pb][:, :, po]
                    tm = tmp.tile([128, NO, BC], f16, tag=tag)
                    nc.vector.scalar_tensor_tensor(
                        tm[:], prev, 0.0, uf[:], MAX, MULT,
                    )
                    nc.vector.tensor_add(cur, tm[:], cur)

            def emit_conv(nb):
                t0, TB = TBLKS[nb]
                hb = h0p.tile([128, NO, TB, BC], g1dt, tag="h0")
                h0blk.append(hb)
                for mg in range(4):
                    sl = slice(mg * 4, (mg + 1) * 4)
                    nc.scalar.activation(hb[:, sl], p0blk[nb][:, sl], RELU)

            def emit_g1(nb):
                t0, TB = TBLKS[nb]
                rb = p1p.tile([128, NO, TB, BC], f16, tag="p1")
                p1blk.append(rb)
                hb = h0blk[nb]
                sc = (1.0 / S1) if FP8 else 1.0
                for m0 in range(0, NO, MG):
                    ps = ps1.tile([128, MG, 16, BC], f32, tag="ps1")
                    for j in range(MG):
                        if FP8:
                            for kp in range(NKP):
                                nc.tensor.matmul(
                                    ps[:, j, :TB],
                                    w1T[:, 2 * kp:2 * kp + 2, m0 + j],
                                    hb[:, 2 * kp:2 * kp + 2],
                                    start=(kp == 0), stop=(kp == NKP - 1),
                                    perf_mode=DR,
                                )
                        else:
                            for k in range(NO):
                                nc.tensor.matmul(
                                    ps[:, j, :TB], w1T[:, k, m0 + j], hb[:, k],
                                    start=(k == 0), stop=(k == NO - 1),
                                )
                    if zero_bias:
                        nc.scalar.activation(
                            rb[:, m0:m0 + MG], ps[:, :, :TB], IDENT,
                            bias=0.0, scale=sc,
                        )
                    else:
                        nc.scalar.activation(
                            rb[:, m0], ps[:, 0, :TB], IDENT,
                            bias=b1t[:, m0:m0 + 1], scale=sc,
                        )

            nblk = len(TBLKS)
            for _rep in range(repeat):
                p0blk.clear()
                h0blk.clear()
                p1blk.clear()
                emit_g0(0)
                emit_g0(1)
                for nb in range(nblk):
                    emit_rec(nb, p0blk, u0f, "tm0")
                    if nb >= 1:
                        emit_conv(nb - 1)
                        emit_g1(nb - 1)
                        emit_rec(nb - 1, p1blk, u1f, "tm1")
                    if nb + 2 < nblk:
                        emit_g0(nb + 2)
                emit_conv(nblk - 1)
                emit_g1(nblk - 1)
                emit_rec(nblk - 1, p1blk, u1f, "tm1")

                # head: out[b] = lin_w . relu(z1_{T-1}) + lin_b
                lb_, lo_ = T2B[T - 1]
                h1h = tmp.tile([128, NO, BC], bf16, tag="h1h")
                nc.scalar.activation(h1h[:], p1blk[lb_][:, :, lo_], RELU)
                ph = ps0.tile([128, MG, 16, BC], f32, tag="ps0")
                for o in range(NO):
                    nc.tensor.matmul(
                        ph[0:1, 0, 0], lwt[:, o:o + 1], h1h[:, o],
                        start=(o == 0), stop=(o == NO - 1),
                    )
                nc.scalar.activation(
                    outs[0:1, :], ph[0:1, 0, 0], IDENT,
                    bias=lbt[0:1, 0:1], scale=1.0,
                )
                nc.sync.dma_start(out=out_d[:], in_=outs[:])

    nc.compile()
    return nc


def _get_nc(zero_bias=True, repeat=1):
    key = ("nc", zero_bias, repeat)
    if key not in _CACHE:
        _CACHE[key] = _build(zero_bias=zero_bias, repeat=repeat)
    return _CACHE[key]


def _trunc22(a):
    return (np.ascontiguousarray(a).view(np.int32) & np.int32(~0x3FF)).view(np.float32)


def _prep_shared(W0, b0, u0, W1, b1, u1, lin_w, lin_b):
    import ml_dtypes

    w0T = np.ascontiguousarray(W0.T).reshape(128, NO, 128).astype(np.float16)
    w1g = W1 * S1 if FP8 else W1
    w1dt = ml_dtypes.float8_e4m3 if FP8 else ml_dtypes.bfloat16
    w1T = np.ascontiguousarray(
        w1g.reshape(NO, 128, NO, 128).transpose(3, 2, 0, 1)
    ).astype(w1dt)
    u0f = np.ascontiguousarray(
        np.broadcast_to(u0.reshape(NO, 128).T[:, :, None], (128, NO, BC))
    ).astype(np.float16)
    u1f = np.ascontiguousarray(
        np.broadcast_to(u1.reshape(NO, 128).T[:, :, None], (128, NO, BC))
    ).astype(np.float16)
    b0t = np.ascontiguousarray(b0.reshape(NO, 128).T)
    b1t = np.ascontiguousarray(b1.reshape(NO, 128).T)
    lwt = np.ascontiguousarray(lin_w.reshape(NO, 128).T).astype(
        ml_dtypes.bfloat16)
    lbt = np.ascontiguousarray(lin_b.reshape(1, 1))
    return dict(w0T=w0T, w1T=w1T, u0f=u0f, u1f=u1f,
                b0t=b0t, b1t=b1t, lwt=lwt, lbt=lbt)


def make_in_maps(x, W0, b0, u0, W1, b1, u1, lin_w, lin_b):
    shared = _prep_shared(
        np.asarray(W0, np.float32), np.asarray(b0, np.float32),
        np.asarray(u0, np.float32), np.asarray(W1, np.float32),
        np.asarray(b1, np.float32), np.asarray(u1, np.float32),
        np.asarray(lin_w, np.float32), np.asarray(lin_b, np.float32),
    )
    x = np.asarray(x, np.float32)
    in_maps = []
    for core in range(NCORES):
        xc = x[core * BL:(core + 1) * BL]            # (BL, T, I)
        xT = np.ascontiguousarray(xc.transpose(2, 1, 0)).astype(np.float16)
        in_maps.append({"xT": xT, **shared})
    return in_maps


def kernel(x, W0, b0, u0, W1, b1, u1, lin_w, lin_b):
    from concourse.bass_utils import run_bass_kernel_spmd

    zb = not (np.any(np.asarray(b0)) or np.any(np.asarray(b1)))
    nc = _get_nc(zero_bias=zb)
    in_maps = make_in_maps(x, W0, b0, u0, W1, b1, u1, lin_w, lin_b)
    try:
        res = run_bass_kernel_spmd(nc, in_maps, list(range(NCORES)))
    except Exception:
        res = run_bass_kernel_spmd(nc, in_maps, list(range(NCORES)))
    return np.concatenate([r["out"][0] for r in res.results])


# revision 62
# speedup vs baseline: 1.4234x; 1.4234x over previous
"""2-layer IndRNN (diagonal recurrence) + linear head on 8 trn2 NeuronCores.

Strategy (data-parallel over batch, 32 rows/core, ONE chunk of BC=32):
  - Feature-major layout [h_inner=partition, (o, t, b)=free]; 16-t blocks.
  - GEMM-0 all-fp16 (x and W0 converted on host), one matmul per (m, block);
    PSUM->SBUF copy on Act fuses bias b0 + fp16 convert into the pre0 ring
    (2-m-tile groups when biases are zero, per-m otherwise).
  - Recurrence keeps the fp16 pre-activation state z_t in place in the pre
    ring, two DVE ops per step:
      tm  = (z_{t-1} max 0) * u   (scalar_tensor_tensor)
      z_t = tm + pre_t            (tensor_tensor add, fp16 2x mode)
  - h0 = relu(z0) -> fp8e4 ring (Act, per-4-m-tile block ops).
  - GEMM-1 in fp8e4 DoubleRow perf mode: 8 k-pair matmuls (2 k-tiles each)
    per m-tile per block; W1 pre-scaled x64 on host; the Act copy applies
    scale=1/64 + bias b1 + fp16 convert into the pre1 ring.
  - Recurrence 1 in place in the pre1 ring; head = relu(z1[T-1]) -> bf16,
    16 accumulated [128,1]x[128,BC] matmuls + lin_b bias.
Host side only reorders/shards numpy inputs; all FLOPs run on device.
"""

import numpy as np

B, T, I, H = 256, 100, 128, 2048
NCORES = 8
BL = B // NCORES            # batch rows per core
BC = BL                     # one chunk
NO = H // 128               # 16 h-tiles
NKP = NO // 2               # 8 fp8 k-pairs
TBLKS = [(0, 4), (4, 8), (12, 16), (28, 16), (44, 16), (60, 16), (76, 16),
         (92, 4), (96, 4)]
T2B = {}
for _nb, (_t0, _tb) in enumerate(TBLKS):
    for _i in range(_tb):
        T2B[_t0 + _i] = (_nb, _i)
S1 = 64.0                   # fp8 weight pre-scale for W1
FP8 = True                  # flip to False for bf16 GEMM-1 fallback

_CACHE = {}


def _build(zero_bias=False, repeat=1):
    import concourse.tile as tile
    from concourse import bacc, mybir

    f32 = mybir.dt.float32
    f16 = mybir.dt.float16
    bf16 = mybir.dt.bfloat16
    f32r = mybir.dt.float32r
    f8 = mybir.dt.float8e4
    g1dt = f8 if FP8 else bf16
    RELU = mybir.ActivationFunctionType.Relu
    IDENT = mybir.ActivationFunctionType.Identity
    DR = mybir.MatmulPerfMode.DoubleRow
    MAX = mybir.AluOpType.max
    MULT = mybir.AluOpType.mult
    # bias==0 for this problem's inputs -> wider PSUM->SBUF copies (the
    # activation bias operand is per-partition, so nonzero per-m biases
    # force per-m copies). Verified at kernel() time; nonzero falls back.
    MG = 2 if zero_bias else 1

    nc = bacc.Bacc(None, target_bir_lowering=False)

    xT_d = nc.dram_tensor("xT", [128, T, BC], f16, kind="ExternalInput")
    w0T_d = nc.dram_tensor("w0T", [128, NO, 128], f16, kind="ExternalInput")
    w1T_d = nc.dram_tensor("w1T", [128, NO, NO, 128], g1dt, kind="ExternalInput")
    u0f_d = nc.dram_tensor("u0f", [128, NO, BC], f16, kind="ExternalInput")
    u1f_d = nc.dram_tensor("u1f", [128, NO, BC], f16, kind="ExternalInput")
    b0_d = nc.dram_tensor("b0t", [128, NO], f32, kind="ExternalInput")
    b1_d = nc.dram_tensor("b1t", [128, NO], f32, kind="ExternalInput")
    lw_d = nc.dram_tensor("lwt", [128, NO], bf16, kind="ExternalInput")
    lb_d = nc.dram_tensor("lbt", [1, 1], f32, kind="ExternalInput")
    out_d = nc.dram_tensor("out", [1, BL], f32, kind="ExternalOutput")

    with tile.TileContext(nc) as tc:
        with (
            tc.tile_pool(name="const", bufs=1) as const,
            tc.tile_pool(name="p0", bufs=4) as p0p,
            tc.tile_pool(name="h0", bufs=3) as h0p,
            tc.tile_pool(name="p1", bufs=3) as p1p,
            tc.tile_pool(name="tmp", bufs=3) as tmp,
            tc.tile_pool(name="ps0", bufs=2, space="PSUM") as ps0,
            tc.tile_pool(name="ps1", bufs=2, space="PSUM") as ps1,
        ):
            xs = const.tile([128, T, BC], f16, tag="xs")
            w0T = const.tile([128, NO, 128], f16, tag="w0T")
            w1T = const.tile([128, NO, NO, 128], g1dt, tag="w1T")
            u0f = const.tile([128, NO, BC], f16, tag="u0f")
            u1f = const.tile([128, NO, BC], f16, tag="u1f")
            b0t = const.tile([128, NO], f32, tag="b0t")
            b1t = const.tile([128, NO], f32, tag="b1t")
            lwt = const.tile([128, NO], bf16, tag="lwt")
            lbt = const.tile([1, 1], f32, tag="lbt")
            outs = const.tile([1, BL], f32, tag="outs")

            # first x block + GEMM-0 weights first so the pipeline starts
            # immediately; bulk x and the large W1 stream behind them.
            t1 = TBLKS[0][1]
            nc.sync.dma_start(out=xs[:, :t1], in_=xT_d[:, :t1])
            nc.sync.dma_start(out=w0T[:], in_=w0T_d[:])
            nc.sync.dma_start(out=u0f[:], in_=u0f_d[:])
            nc.sync.dma_start(out=b0t[:], in_=b0_d[:])
            nc.sync.dma_start(out=xs[:, t1:], in_=xT_d[:, t1:])
            for kb in range(NO):
                nc.sync.dma_start(out=w1T[:, kb], in_=w1T_d[:, kb])
            nc.sync.dma_start(out=u1f[:], in_=u1f_d[:])
            nc.sync.dma_start(out=b1t[:], in_=b1_d[:])
            nc.sync.dma_start(out=lwt[:], in_=lw_d[:])
            nc.sync.dma_start(out=lbt[:], in_=lb_d[:])

            p0blk = []
            h0blk = []
            p1blk = []

            def emit_g0(nb):
                t0, TB = TBLKS[nb]
                pb = p0p.tile([128, NO, TB, BC], f16, tag="p0")
                p0blk.append(pb)
                # small ramp blocks: 4-m-tile groups fit the same 2-bank
                # PSUM footprint, halving copy count on the critical lead-in
                mg = 4 if (zero_bias and TB <= 8) else MG
                for m0 in range(0, NO, mg):
                    ps = ps0.tile([128, mg, 16 * MG // mg, BC], f32, tag="ps0")
                    for j in range(mg):
                        nc.tensor.matmul(
                            ps[:, j, :TB], w0T[:, m0 + j], xs[:, t0:t0 + TB],
                            start=True, stop=True,
                        )
                    if zero_bias:
                        nc.scalar.activation(
                            pb[:, m0:m0 + mg], ps[:, :, :TB], IDENT,
                            bias=0.0, scale=1.0,
                        )
                    else:
                        nc.scalar.activation(
                            pb[:, m0], ps[:, 0, :TB], IDENT,
                            bias=b0t[:, m0:m0 + 1], scale=1.0,
                        )

            def emit_rec(nb, blks, uf, tag):
                t0, TB = TBLKS[nb]
                for trel in range(TB):
                    t = t0 + trel
                    if t == 0:
                        continue
                    cur = blks[nb][:, :, trel]
                    pb, po = T2B[t - 1]
                    prev = blks[pb][:, :, po]
                    tm = tmp.tile([128, NO, BC], f16, tag=tag)
                    nc.vector.scalar_tensor_tensor(
                        tm[:], prev, 0.0, uf[:], MAX, MULT,
                    )
                    nc.vector.tensor_add(cur, tm[:], cur)

            def emit_conv(nb):
                t0, TB = TBLKS[nb]
                hb = h0p.tile([128, NO, TB, BC], g1dt, tag="h0")
                h0blk.append(hb)
                for mg in range(4):
                    sl = slice(mg * 4, (mg + 1) * 4)
                    nc.scalar.activation(hb[:, sl], p0blk[nb][:, sl], RELU)

            def emit_g1(nb):
                t0, TB = TBLKS[nb]
                rb = p1p.tile([128, NO, TB, BC], f16, tag="p1")
                p1blk.append(rb)
                hb = h0blk[nb]
                sc = (1.0 / S1) if FP8 else 1.0
                mg = 4 if (zero_bias and TB <= 8) else MG
                for m0 in range(0, NO, mg):
                    ps = ps1.tile([128, mg, 16 * MG // mg, BC], f32, tag="ps1")
                    for j in range(mg):
                        if FP8:
                            for kp in range(NKP):
                                nc.tensor.matmul(
                                    ps[:, j, :TB],
                                    w1T[:, 2 * kp:2 * kp + 2, m0 + j],
                                    hb[:, 2 * kp:2 * kp + 2],
                                    start=(kp == 0), stop=(kp == NKP - 1),
                                    perf_mode=DR,
                                )
                        else:
                            for k in range(NO):
                                nc.tensor.matmul(
                                    ps[:, j, :TB], w1T[:, k, m0 + j], hb[:, k],
                                    start=(k == 0), stop=(k == NO - 1),
                                )
                    if zero_bias:
                        nc.scalar.activation(
                            rb[:, m0:m0 + mg], ps[:, :, :TB], IDENT,
                            bias=0.0, scale=sc,
                        )
                    else:
                        nc.scalar.activation(
                            rb[:, m0], ps[:, 0, :TB], IDENT,
                            bias=b1t[:, m0:m0 + 1], scale=sc,
                        )

            nblk = len(TBLKS)
            for _rep in range(repeat):
                p0blk.clear()
                h0blk.clear()
                p1blk.clear()
                emit_g0(0)
                emit_g0(1)
                for nb in range(nblk):
                    emit_rec(nb, p0blk, u0f, "tm0")
                    if nb >= 1:
                        emit_conv(nb - 1)
                        emit_g1(nb - 1)
                        emit_rec(nb - 1, p1blk, u1f, "tm1")
                    if nb + 2 < nblk:
                        emit_g0(nb + 2)
                emit_conv(nblk - 1)
                emit_g1(nblk - 1)
                emit_rec(nblk - 1, p1blk, u1f, "tm1")

                # head: out[b] = lin_w . relu(z1_{T-1}) + lin_b
                lb_, lo_ = T2B[T - 1]
                h1h = tmp.tile([128, NO, BC], bf16, tag="h1h")
                nc.scalar.activation(h1h[:], p1blk[lb_][:, :, lo_], RELU)
                ph = ps0.tile([128, MG, 16, BC], f32, tag="ps0")
                for o in range(NO):
                    nc.tensor.matmul(
                        ph[0:1, 0, 0], lwt[:, o:o + 1], h1h[:, o],
                        start=(o == 0), stop=(o == NO - 1),
                    )
                nc.scalar.activation(
                    outs[0:1, :], ph[0:1, 0, 0], IDENT,
                    bias=lbt[0:1, 0:1], scale=1.0,
                )
                nc.sync.dma_start(out=out_d[:], in_=outs[:])

    nc.compile()
    return nc


def _get_nc(zero_bias=True, repeat=1):
    key = ("nc", zero_bias, repeat)
    if key not in _CACHE:
        _CACHE[key] = _build(zero_bias=zero_bias, repeat=repeat)
    return _CACHE[key]


def _trunc22(a):
    return (np.ascontiguousarray(a).view(np.int32) & np.int32(~0x3FF)).view(np.float32)


def _prep_shared(W0, b0, u0, W1, b1, u1, lin_w, lin_b):
    import ml_dtypes

    w0T = np.ascontiguousarray(W0.T).reshape(128, NO, 128).astype(np.float16)
    w1g = W1 * S1 if FP8 else W1
    w1dt = ml_dtypes.float8_e4m3 if FP8 else ml_dtypes.bfloat16
    w1T = np.ascontiguousarray(
        w1g.reshape(NO, 128, NO, 128).transpose(3, 2, 0, 1)
    ).astype(w1dt)
    u0f = np.ascontiguousarray(
        np.broadcast_to(u0.reshape(NO, 128).T[:, :, None], (128, NO, BC))
    ).astype(np.float16)
    u1f = np.ascontiguousarray(
        np.broadcast_to(u1.reshape(NO, 128).T[:, :, None], (128, NO, BC))
    ).astype(np.float16)
    b0t = np.ascontiguousarray(b0.reshape(NO, 128).T)
    b1t = np.ascontiguousarray(b1.reshape(NO, 128).T)
    lwt = np.ascontiguousarray(lin_w.reshape(NO, 128).T).astype(
        ml_dtypes.bfloat16)
    lbt = np.ascontiguousarray(lin_b.reshape(1, 1))
    return dict(w0T=w0T, w1T=w1T, u0f=u0f, u1f=u1f,
                b0t=b0t, b1t=b1t, lwt=lwt, lbt=lbt)


def make_in_maps(x, W0, b0, u0, W1, b1, u1, lin_w, lin_b):
    shared = _prep_shared(
        np.asarray(W0, np.float32), np.asarray(b0, np.float32),
        np.asarray(u0, np.float32), np.asarray(W1, np.float32),
        np.asarray(b1, np.float32), np.asarray(u1, np.float32),
        np.asarray(lin_w, np.float32), np.asarray(lin_b, np.float32),
    )
    x = np.asarray(x, np.float32)
    in_maps = []
    for core in range(NCORES):
        xc = x[core * BL:(core + 1) * BL]            # (BL, T, I)
        xT = np.ascontiguousarray(xc.transpose(2, 1, 0)).astype(np.float16)
        in_maps.append({"xT": xT, **shared})
    return in_maps


def kernel(x, W0, b0, u0, W1, b1, u1, lin_w, lin_b):
    from concourse.bass_utils import run_bass_kernel_spmd

    zb = not (np.any(np.asarray(b0)) or np.any(np.asarray(b1)))
    nc = _get_nc(zero_bias=zb)
    in_maps = make_in_maps(x, W0, b0, u0, W1, b1, u1, lin_w, lin_b)
    try:
        res = run_bass_kernel_spmd(nc, in_maps, list(range(NCORES)))
    except Exception:
        res = run_bass_kernel_spmd(nc, in_maps, list(range(NCORES)))
    return np.concatenate([r["out"][0] for r in res.results])
